# revision 1
# baseline (speedup 1.0000x reference)
"""MFDWC feature extractor as a Bass/Tile kernel for TRN2 (8 NeuronCores).

Pipeline (per batch row): pre-emphasis -> framing (999 frames x 882 samples,
hop 441) -> Hamming window -> rFFT(2048) power spectrum -> mel (60) -> log ->
Haar DWT -> delta -> mean/std over time -> 180 features.

Device mapping:
  - Data parallel: 16 batch rows -> 2 rows per core on 8 cores.
  - rFFT is computed as two DFT matmuls (cos / sin matrices, window folded in)
    in fp16 at full PE rate, fp32 PSUM accumulation.
  - The waveform is reshaped on-chip to put the sample-within-frame axis on
    SBUF partitions (PE transposes of 441-wide chunks); frames then appear as
    overlapping column views of a single (441, 1000) buffer.
  - Bins packing: cos matmul covers bins 0..1023; the sin matrix's bin-0
    column (which would be all zeros) instead carries the Nyquist cos column,
    and the two mel matrices are adjusted to match, so all 1025 power bins are
    covered by 2x1024 columns with no waste.
  - power -> mel is a second (tiny) matmul pair; log/Haar/delta/stats run on
    ACT/DVE engines.
"""

import math
from contextlib import ExitStack

import numpy as np

import concourse.bass as bass
import concourse.bacc as bacc
import concourse.mybir as mybir
import concourse.tile as tile
from concourse.bass_utils import run_bass_kernel_spmd

F32 = mybir.dt.float32
F16 = mybir.dt.float16
AF = mybir.ActivationFunctionType

B = 16               # batch
L = 441000           # samples per row
W = 441              # hop; also chunk width
NK = 1000            # number of 441-sample chunks per row (441*1000 = L)
FRAME = 882          # frame length
T = 999              # frames per row
NB = 1024            # matmul bins (bins 0..1023; Nyquist packed into sin col 0)
NMEL = 60
ROWS = 2             # batch rows per core
EPS = 1e-10
SQRT2 = math.sqrt(2.0)

# contraction chunks over the 882 frame samples: (r0, size, a) where the
# frame-sample index j = 441*a + r0 + i
KCH = [(0, 128, 0), (128, 128, 0), (256, 128, 0), (384, 57, 0),
       (0, 128, 1), (128, 128, 1), (256, 128, 1), (384, 57, 1)]
# chunks over the NK=1000 waveform rows
ECH = [(k * 128, min(128, NK - k * 128)) for k in range(8)]
# transpose row-blocks over the 441 samples per chunk
RBL = [(0, 128), (128, 128), (256, 128), (384, 57)]
# frame chunks (PSUM free-dim <= 512 fp32)
FCH = [(0, 512), (512, 487)]


def _host_constants(mel_filters: np.ndarray):
    """DFT / mel matrices with window folded in (fp16)."""
    j = np.arange(FRAME, dtype=np.float64)
    b = np.arange(NB, dtype=np.float64)
    ham = np.hamming(FRAME).astype(np.float64)
    ang = 2.0 * np.pi * np.outer(j, b) / 2048.0
    cw = (ham[:, None] * np.cos(ang)).astype(np.float16)          # (882, 1024)
    sw = ham[:, None] * np.sin(ang)
    sw[:, 0] = ham * np.cos(np.pi * j)                            # Nyquist cos col
    sw = sw.astype(np.float16)                                    # (882, 1024)
    m = mel_filters.astype(np.float64)                            # (60, 1025)
    mat = m[:, 0:NB].T.astype(np.float16)                         # (1024, 60)
    mbt = np.concatenate([m[:, NB:NB + 1], m[:, 1:NB]], axis=1).T.astype(np.float16)
    idn = np.eye(128, dtype=np.float16)
    hsum = np.zeros((NMEL, 30), np.float16)
    hdif = np.zeros((NMEL, 30), np.float16)
    for i in range(30):
        hsum[2 * i, i] = 1.0
        hsum[2 * i + 1, i] = 1.0
        hdif[2 * i, i] = 1.0
        hdif[2 * i + 1, i] = -1.0
    return cw, sw, mat, mbt, idn, hsum, hdif


def _body(ctx: ExitStack, tc, xpad, cw_d, sw_d, mat_d, mbt_d, idn_d, hs_d, hd_d, out_d):
    nc = tc.nc

    const = ctx.enter_context(tc.tile_pool(name="const", bufs=1))
    e2p = ctx.enter_context(tc.tile_pool(name="e2", bufs=3))
    emphp = ctx.enter_context(tc.tile_pool(name="emph", bufs=3))
    etp = ctx.enter_context(tc.tile_pool(name="et", bufs=1))
    ptrp = ctx.enter_context(tc.tile_pool(name="ptr", bufs=1, space="PSUM"))
    dftp = ctx.enter_context(tc.tile_pool(name="dft", bufs=2, space="PSUM"))
    melp = ctx.enter_context(tc.tile_pool(name="mel", bufs=1, space="PSUM"))
    haarp = ctx.enter_context(tc.tile_pool(name="haar", bufs=1, space="PSUM"))
    ppp = ctx.enter_context(tc.tile_pool(name="pp", bufs=2))
    lmp = ctx.enter_context(tc.tile_pool(name="lm", bufs=1))
    hop = ctx.enter_context(tc.tile_pool(name="ho", bufs=1))
    stp = ctx.enter_context(tc.tile_pool(name="st", bufs=2))

    # constants
    cw_t, sw_t = [], []
    for ki, (r0, sz, a) in enumerate(KCH):
        j0 = 441 * a + r0
        t = const.tile([128, NB], F16, tag=f"cw{ki}", name=f"cw{ki}")
        nc.sync.dma_start(t[0:sz, :], cw_d[j0:j0 + sz, :])
        cw_t.append(t)
        t = const.tile([128, NB], F16, tag=f"sw{ki}", name=f"sw{ki}")
        nc.sync.dma_start(t[0:sz, :], sw_d[j0:j0 + sz, :])
        sw_t.append(t)
    mat_t, mbt_t = [], []
    for c in range(8):
        t = const.tile([128, NMEL], F16, tag=f"ma{c}", name=f"ma{c}")
        nc.sync.dma_start(t[:, :], mat_d[c * 128:(c + 1) * 128, :])
        mat_t.append(t)
        t = const.tile([128, NMEL], F16, tag=f"mb{c}", name=f"mb{c}")
        nc.sync.dma_start(t[:, :], mbt_d[c * 128:(c + 1) * 128, :])
        mbt_t.append(t)
    ident = const.tile([128, 128], F16, tag="id", name="ident")
    nc.sync.dma_start(ident[:, :], idn_d[:, :])
    eps_t = const.tile([128, 1], F32, tag="eps", name="eps")
    nc.vector.memset(eps_t[:, :], EPS)
    hs_t = const.tile([NMEL, 30], F16, tag="hs", name="hs")
    nc.sync.dma_start(hs_t[:, :], hs_d[:, :])
    hd_t = const.tile([NMEL, 30], F16, tag="hd", name="hd")
    nc.sync.dma_start(hd_t[:, :], hd_d[:, :])

    for r in range(ROWS):
        # ---- phase 1: pre-emphasis + on-chip transpose to (441, 1000) fp16
        et = [etp.tile([128, NK], F16, tag=f"et{r}_{c}", name=f"et{r}_{c}") for c in range(4)]
        for (k0, ksz) in ECH:
            e2 = e2p.tile([128, W + 1], F32, tag="e2", name="e2")
            src = bass.AP(xpad, r * (L + 1) + W * k0, [[W, ksz], [1, W + 1]])
            nc.sync.dma_start(e2[0:ksz, :], src)
            tmp = emphp.tile([128, W], F16, tag="tmp", name="tmp")
            nc.scalar.mul(tmp[0:ksz, :], e2[0:ksz, 0:W], 0.97)
            em = emphp.tile([128, W], F16, tag="em", name="em")
            nc.vector.tensor_sub(em[0:ksz, :], e2[0:ksz, 1:W + 1], tmp[0:ksz, :])
            for rb, (rb0, rbsz) in enumerate(RBL):
                ptr = ptrp.tile([128, 128], F16, tag="ptr", name="ptr")
                nc.tensor.transpose(ptr[0:rbsz, 0:ksz], em[0:ksz, rb0:rb0 + rbsz],
                                    ident[0:ksz, 0:ksz])
                nc.scalar.copy(et[rb][0:rbsz, k0:k0 + ksz], ptr[0:rbsz, 0:ksz])

        # ---- phase 2: DFT power -> mel -> log
        lm = lmp.tile([NMEL, T], F16, tag=f"lm{r}", name=f"lm{r}")
        for (f0, fN) in FCH:
            mp = melp.tile([NMEL, 512], F32, tag="mp", name="mp")
            for bc in range(8):
                pre = dftp.tile([128, 512], F32, tag="pre", name="pre")
                pim = dftp.tile([128, 512], F32, tag="pim", name="pim")
                for ki, (r0, sz, a) in enumerate(KCH):
                    rhs = et[r0 // 128][0:sz, f0 + a:f0 + a + fN]
                    nc.tensor.matmul(pre[:, 0:fN], cw_t[ki][0:sz, bc * 128:(bc + 1) * 128],
                                     rhs, start=(ki == 0), stop=(ki == 7))
                for ki, (r0, sz, a) in enumerate(KCH):
                    rhs = et[r0 // 128][0:sz, f0 + a:f0 + a + fN]
                    nc.tensor.matmul(pim[:, 0:fN], sw_t[ki][0:sz, bc * 128:(bc + 1) * 128],
                                     rhs, start=(ki == 0), stop=(ki == 7))
                pa = ppp.tile([128, 512], F16, tag="pa", name="pa")
                nc.scalar.square(pa[:, 0:fN], pre[:, 0:fN])
                pb = ppp.tile([128, 512], F16, tag="pb", name="pb")
                nc.scalar.square(pb[:, 0:fN], pim[:, 0:fN])
                nc.tensor.matmul(mp[0:NMEL, 0:fN], mat_t[bc][:, 0:NMEL], pa[:, 0:fN],
                                 start=(bc == 0), stop=False, skip_group_check=True)
                nc.tensor.matmul(mp[0:NMEL, 0:fN], mbt_t[bc][:, 0:NMEL], pb[:, 0:fN],
                                 start=False, stop=(bc == 7), skip_group_check=True)
            nc.scalar.activation(lm[0:NMEL, f0:f0 + fN], mp[0:NMEL, 0:fN], AF.Ln,
                                 bias=eps_t[0:NMEL, :])

        # ---- phase 3: Haar (as tiny matmuls) / delta / stats
        ca = hop.tile([30, T], F32, tag=f"ca{r}", name=f"ca{r}")
        cd = hop.tile([30, T], F32, tag=f"cd{r}", name=f"cd{r}")
        for (f0, fN) in FCH:
            pca = haarp.tile([30, 512], F32, tag="pca", name="pca")
            nc.tensor.matmul(pca[:, 0:fN], hs_t[:, :], lm[0:NMEL, f0:f0 + fN],
                             start=True, stop=True, skip_group_check=True)
            nc.scalar.copy(ca[:, f0:f0 + fN], pca[:, 0:fN])
            pcd = haarp.tile([30, 512], F32, tag="pcd", name="pcd")
            nc.tensor.matmul(pcd[:, 0:fN], hd_t[:, :], lm[0:NMEL, f0:f0 + fN],
                             start=True, stop=True, skip_group_check=True)
            nc.scalar.copy(cd[:, f0:f0 + fN], pcd[:, 0:fN])
        dl = hop.tile([30, T], F32, tag=f"dl{r}", name=f"dl{r}")
        nc.vector.tensor_sub(dl[:, 1:T - 1], ca[:, 2:T], ca[:, 0:T - 2])
        nc.vector.tensor_sub(dl[:, 0:1], ca[:, 1:2], ca[:, 0:1])
        nc.vector.tensor_sub(dl[:, T - 1:T], ca[:, T - 1:T], ca[:, T - 2:T - 1])

        stats = stp.tile([30, 6], F32, tag=f"stats{r}", name=f"stats{r}")
        for si, feat in enumerate((ca, dl, cd)):
            s1 = stp.tile([30, 1], F32, tag="s1", name="s1")
            nc.vector.tensor_reduce(s1[:, :], feat[:, :], axis=mybir.AxisListType.X,
                                    op=mybir.AluOpType.add)
            nc.vector.tensor_scalar_mul(stats[:, si:si + 1], s1[:, :], 1.0 / (T * SQRT2))
            nm = stp.tile([30, 1], F32, tag="nm", name="nm")
            nc.vector.tensor_scalar_mul(nm[:, :], s1[:, :], -1.0 / T)
            scr = stp.tile([30, T], F32, tag="scr", name="scr")
            sq = stp.tile([30, 1], F32, tag="sq", name="sq")
            nc.scalar.activation(scr[:, :], feat[:, :], AF.Square, bias=nm[:, :],
                                 scale=1.0, accum_out=sq[:, :])
            nc.scalar.activation(stats[:, 3 + si:4 + si], sq[:, :], AF.Sqrt,
                                 scale=1.0 / ((T - 1) * 2.0))
        nc.sync.dma_start(bass.AP(out_d, r * 180, [[1, 180]]), stats[:, :])


_CACHE = {}


def _build():
    if "nc" in _CACHE:
        return _CACHE["nc"]
    nc = bacc.Bacc("TRN2", target_bir_lowering=False, debug=False,
                   enable_asserts=False, num_devices=8)
    xpad = nc.dram_tensor("xpad", [ROWS, L + 1], F32, kind="ExternalInput")
    cw_d = nc.dram_tensor("cw", [FRAME, NB], F16, kind="ExternalInput")
    sw_d = nc.dram_tensor("sw", [FRAME, NB], F16, kind="ExternalInput")
    mat_d = nc.dram_tensor("mat", [NB, NMEL], F16, kind="ExternalInput")
    mbt_d = nc.dram_tensor("mbt", [NB, NMEL], F16, kind="ExternalInput")
    idn_d = nc.dram_tensor("idn", [128, 128], F16, kind="ExternalInput")
    hs_d = nc.dram_tensor("hsum", [NMEL, 30], F16, kind="ExternalInput")
    hd_d = nc.dram_tensor("hdif", [NMEL, 30], F16, kind="ExternalInput")
    out_d = nc.dram_tensor("out", [ROWS, 180], F32, kind="ExternalOutput")
    with tile.TileContext(nc) as tc, ExitStack() as ctx:
        _body(ctx, tc, xpad, cw_d, sw_d, mat_d, mbt_d, idn_d, hs_d, hd_d, out_d)
    nc.compile()
    _CACHE["nc"] = nc
    return nc


def make_in_maps(waveform: np.ndarray, mel_filters: np.ndarray):
    cw, sw, mat, mbt, idn, hsum, hdif = _host_constants(mel_filters)
    in_maps = []
    for core in range(8):
        rows = waveform[ROWS * core:ROWS * (core + 1)]
        xpad = np.zeros((ROWS, L + 1), np.float32)
        xpad[:, 1:] = rows
        in_maps.append({"xpad": xpad, "cw": cw, "sw": sw, "mat": mat,
                        "mbt": mbt, "idn": idn, "hsum": hsum, "hdif": hdif})
    return in_maps


def gather_out(results):
    # device rows are packed [mel_idx, stat]; reorder to [stat, mel_idx]
    full = np.concatenate([results[c]["out"] for c in range(8)], axis=0)
    return np.ascontiguousarray(
        full.reshape(B, 30, 6).transpose(0, 2, 1).reshape(B, 180)).astype(np.float32)


def run(waveform, mel_filters, trace=False):
    nc = _build()
    in_maps = make_in_maps(np.asarray(waveform, np.float32),
                           np.asarray(mel_filters, np.float32))
    res = run_bass_kernel_spmd(nc, in_maps, core_ids=list(range(8)), trace=trace)
    return gather_out(res.results), res


def kernel(waveform: np.ndarray, mel_filters: np.ndarray) -> np.ndarray:
    out, _ = run(waveform, mel_filters, trace=False)
    return out

